# revision 1
# baseline (speedup 1.0000x reference)
"""MoE (top-2 routing, E=8 experts) Trainium2 kernel.

Strategy (expert-parallel across 8 NeuronCores):
  - Host: gate (tiny: [2048,1024]@[1024,8]) in fp64, top-2 + softmax exactly
    reproducing the reference routing (selection gaps are ~4e-4, far above
    fp32 noise, so fp64 routing == reference fp32 routing).
  - Host: dispatch — gather each expert's tokens into a fixed-capacity,
    zero-padded buffer; one expert per core. Weights are pre-transposed,
    cast to bf16 and packed per matmul-group on host so each core's full
    working set (w1.T 8MiB + w2.T 8MiB bf16) is SBUF-resident and every
    DMA is contiguous per partition and arrives in consumption order.
  - Device (per core): h.T = gelu(w1 @ x.T) [F x C], y.T = w2 @ h.T [D x C],
    tokens live in the matmul free dimension. Routing coefficient is applied
    as part of the PSUM->SBUF eviction (DVE multiply). fp32 accumulation in
    PSUM throughout.
  - Host: combine — scatter-add the two expert contributions per token.
"""

import numpy as np
import ml_dtypes

import sys

if "/opt/trn_rl_repo" not in sys.path:
    sys.path.insert(0, "/opt/trn_rl_repo")

import concourse.tile as tile
from concourse import bacc, mybir
from concourse.bass_utils import run_bass_kernel_spmd

BF16 = ml_dtypes.bfloat16

E, D, F, NTOK = 8, 1024, 4096, 2048
P = 128
KD, KF = D // P, F // P  # 8, 32

_NC_CACHE: dict = {}


def _plan_capacity(max_count: int):
    """Pick capacity C = n_chunks * L with L <= 512, L a multiple of 16."""
    max_count = max(max_count, 16)
    n = -(-max_count // 512)  # ceil
    L = -(-max_count // (n * 16)) * 16
    return n * L, L, n


def _build_nc(C: int, L: int):
    nchunks = C // L
    # Bacc (not raw Bass): its finalize() runs move_matmul_waits_to_ldweights
    # + generate_event_semaphores, which split multi-wait instructions down to
    # the TRN2 limit of 1 sync wait per instruction.
    nc = bacc.Bacc(None)
    # x packed per chunk:   x_pack[c, p, k, t]  = x_e.T[k*P + p, c*L + t]
    # w1 packed per f-tile: w1_pack[f, p, k, c] = w1[e][f*P + c, k*P + p]
    # w2 packed per d-tile: w2_pack[d, p, k2, c] = w2[e][d*P + c, k2*P + p]
    x_in = nc.declare_dram_parameter("x_pack", [nchunks, P, KD, L], mybir.dt.bfloat16, isOutput=False)
    w1_in = nc.declare_dram_parameter("w1_pack", [KF, P, KD, P], mybir.dt.bfloat16, isOutput=False)
    w2_in = nc.declare_dram_parameter("w2_pack", [KD, P, KF, P], mybir.dt.bfloat16, isOutput=False)
    cf_in = nc.declare_dram_parameter("coef", [P, C], mybir.dt.float32, isOutput=False)
    y_out = nc.declare_dram_parameter("y_dc", [KD, P, C], mybir.dt.float32, isOutput=True)

    with tile.TileContext(nc) as tc:
        with (
            tc.tile_pool(name="wpool", bufs=1) as wpool,
            tc.tile_pool(name="apool", bufs=1) as apool,
            tc.tile_pool(name="ps1", bufs=4, space="PSUM") as ps1,
            tc.tile_pool(name="ps2", bufs=3, space="PSUM") as ps2,
            tc.tile_pool(name="psw", bufs=1, space="PSUM") as psw,
            tc.tile_pool(name="ypool", bufs=6) as ypool,
        ):
            # ---- PE warm-up: ~4us of dependency-free matmuls run during the
            # input-DMA head so the HAM clock gate is at 8/8 (2.4 GHz) when
            # the real stream starts (saves ~3us of cold-clock matmuls).
            warm = wpool.tile([P, P], mybir.dt.bfloat16, name="warm")
            nc.vector.memset(warm, 0.0)
            N_WARM = 112
            ps_w = psw.tile([P, 64], mybir.dt.float32, name="ps_w")
            for i in range(N_WARM):
                nc.tensor.matmul(
                    ps_w, warm, warm[:, :64], start=(i == 0), stop=(i == N_WARM - 1)
                )
            # ---- loads, in consumption order, one DMA per consumer group.
            # w1_0 first, then x chunks, then remaining w1: the first S1 group
            # needs w1_0 + x_0 and everything lands on one HWDGE queue in
            # program order.
            w1_sb = [None] * KF
            x_sb = [None] * nchunks

            def load_w1(f):
                wt = wpool.tile([P, KD, P], mybir.dt.bfloat16, name=f"w1_{f}")
                nc.sync.dma_start(wt, w1_in[f])
                w1_sb[f] = wt

            load_w1(0)
            for c in range(nchunks):
                xt = apool.tile([P, KD, L], mybir.dt.bfloat16, name=f"x_{c}")
                nc.sync.dma_start(xt, x_in[c])
                x_sb[c] = xt
            for f in range(1, KF):
                load_w1(f)
            coef_sb = apool.tile([P, C], mybir.dt.float32, name="coef_sb")
            nc.sync.dma_start(coef_sb, cf_in[:])
            w2_sb = []
            for d in range(KD):
                wt = wpool.tile([P, KF, P], mybir.dt.bfloat16, name=f"w2_{d}")
                nc.sync.dma_start(wt, w2_in[d])
                w2_sb.append(wt)

            h_sb = [
                apool.tile([P, C], mybir.dt.bfloat16, name=f"h_{k2}") for k2 in range(KF)
            ]

            # ---- stage 1: h.T[f*P:(f+1)*P, :] = gelu(w1 @ x.T) per F-tile
            for f in range(KF):
                for c in range(nchunks):
                    c0 = c * L
                    ps = ps1.tile([P, L], mybir.dt.float32, name="ps1t", tag="ps1t")
                    for k in range(KD):
                        nc.tensor.matmul(
                            ps,
                            w1_sb[f][:, k],
                            x_sb[c][:, k],
                            start=(k == 0),
                            stop=(k == KD - 1),
                        )
                    nc.scalar.activation(
                        out=h_sb[f][:, c0 : c0 + L],
                        in_=ps,
                        func=mybir.ActivationFunctionType.Gelu,
                    )

            # ---- stage 2: y.T[d*P:(d+1)*P, :] = (w2 @ h.T) * coef
            for d in range(KD):
                for c in range(nchunks):
                    c0 = c * L
                    ps = ps2.tile([P, L], mybir.dt.float32, name="ps2t", tag="ps2t")
                    for k2 in range(KF):
                        nc.tensor.matmul(
                            ps,
                            w2_sb[d][:, k2],
                            h_sb[k2][:, c0 : c0 + L],
                            start=(k2 == 0),
                            stop=(k2 == KF - 1),
                        )
                    y_sb = ypool.tile([P, L], mybir.dt.float32, name="y_sb", tag="y_sb")
                    nc.vector.tensor_mul(y_sb, ps, coef_sb[:, c0 : c0 + L])
                    nc.sync.dma_start(y_out[d][:, c0 : c0 + L], y_sb)
    nc.finalize()
    return nc


def _route(x: np.ndarray, gate_w: np.ndarray):
    """fp64 gating; matches reference fp32 routing (selection gaps >> fp32 eps)."""
    logits = x.astype(np.float64) @ gate_w.astype(np.float64).T  # [N, E]
    top2 = np.argsort(-logits, axis=1, kind="stable")[:, :2]  # [N, 2]
    v = np.take_along_axis(logits, top2, axis=1)
    v = v - v.max(axis=1, keepdims=True)
    ew = np.exp(v)
    w = ew / ew.sum(axis=1, keepdims=True)  # [N, 2]
    return top2, w.astype(np.float32)


def _run(inputs: dict, trace: bool = False, trace_cores=None):
    x = np.asarray(inputs["x"], dtype=np.float32)
    gate_w = np.asarray(inputs["gate_w"], dtype=np.float32)
    w1 = np.asarray(inputs["w1"], dtype=np.float32)
    w2 = np.asarray(inputs["w2"], dtype=np.float32)
    n = x.shape[0]

    top2, wsm = _route(x, gate_w)

    idx_list, coef_list = [], []
    for e in range(E):
        mask = top2 == e  # [N, 2]
        sel = mask.any(axis=1)
        idx = np.nonzero(sel)[0]
        we = np.where(mask[idx, 0], wsm[idx, 0], wsm[idx, 1])
        idx_list.append(idx)
        coef_list.append(we.astype(np.float32))

    max_count = max(len(i) for i in idx_list)
    C, L, nchunks = _plan_capacity(max_count)

    key = (C, L)
    if key not in _NC_CACHE:
        _NC_CACHE[key] = _build_nc(C, L)
    nc = _NC_CACHE[key]

    in_maps = []
    for e in range(E):
        idx, cf = idx_list[e], coef_list[e]
        cnt = len(idx)
        xe = np.zeros((D, C), dtype=BF16)
        xe[:, :cnt] = x[idx].T.astype(BF16)
        # [D, C] -> [k, p, c, t] -> pack [c, p, k, t]
        x_pack = np.ascontiguousarray(
            xe.reshape(KD, P, nchunks, L).transpose(2, 1, 0, 3)
        )
        coef = np.zeros((C,), dtype=np.float32)
        coef[:cnt] = cf
        coef_rep = np.ascontiguousarray(np.broadcast_to(coef, (P, C)))
        # w1[e] is [F, D]: [f, c, k, p] -> pack [f, p, k, c]
        w1_pack = np.ascontiguousarray(
            w1[e].astype(BF16).reshape(KF, P, KD, P).transpose(0, 3, 2, 1)
        )
        # w2[e] is [D, F]: [d, c, k2, p] -> pack [d, p, k2, c]
        w2_pack = np.ascontiguousarray(
            w2[e].astype(BF16).reshape(KD, P, KF, P).transpose(0, 3, 2, 1)
        )
        in_maps.append(
            {
                "x_pack": x_pack,
                "w1_pack": w1_pack,
                "w2_pack": w2_pack,
                "coef": coef_rep,
            }
        )

    res = run_bass_kernel_spmd(
        nc,
        in_maps,
        list(range(E)),
        trace=trace,
        trace_cores=trace_cores,
    )

    out = np.zeros((n, D), dtype=np.float32)
    for e in range(E):
        idx = idx_list[e]
        cnt = len(idx)
        y_dc = np.asarray(res.results[e]["y_dc"], dtype=np.float32)  # [KD, P, C]
        y = y_dc.reshape(D, C)[:, :cnt]  # [D, cnt]
        out[idx] += y.T
    return out, res


def kernel(**inputs) -> np.ndarray:
    out, _ = _run(inputs, trace=False)
    return out


if __name__ == "__main__":
    rng = np.random.default_rng(0)
    fake = {
        "x": rng.standard_normal((NTOK, D), dtype=np.float32),
        "gate_w": (rng.standard_normal((E, D)) * 0.02).astype(np.float32),
        "w1": (rng.standard_normal((E, F, D)) * 0.02).astype(np.float32),
        "w2": (rng.standard_normal((E, D, F)) * 0.02).astype(np.float32),
    }
    out = kernel(**fake)
    print("ok", out.shape, out.dtype, np.abs(out).max())



# revision 6
# speedup vs baseline: 1.3435x; 1.3435x over previous
"""MoE (top-2 routing, E=8 experts) Trainium2 kernel.

Strategy (expert-parallel across 8 NeuronCores):
  - Host: gate (tiny: [2048,1024]@[1024,8]) in fp64, top-2 + softmax exactly
    reproducing the reference routing (selection gaps are ~4e-4, far above
    fp32 noise, so fp64 routing == reference fp32 routing).
  - Host: dispatch — gather each expert's tokens into a fixed-capacity,
    zero-padded buffer; one expert per core. Weights are pre-transposed,
    cast to bf16 and packed per matmul-group on host so each core's full
    working set (w1.T 8MiB + w2.T 8MiB bf16) is SBUF-resident and every
    DMA is contiguous per partition and arrives in consumption order.
  - Device (per core): h.T = gelu(w1 @ x.T) [F x C], y.T = w2 @ h.T [D x C],
    tokens live in the matmul free dimension. Routing coefficient is applied
    as part of the PSUM->SBUF eviction (DVE multiply). fp32 accumulation in
    PSUM throughout.
  - Host: combine — scatter-add the two expert contributions per token.
"""

import numpy as np
import ml_dtypes

import sys

if "/opt/trn_rl_repo" not in sys.path:
    sys.path.insert(0, "/opt/trn_rl_repo")

import concourse.tile as tile
from concourse import bacc, mybir
from concourse.bass_utils import run_bass_kernel_spmd

BF16 = ml_dtypes.bfloat16

E, D, F, NTOK = 8, 1024, 4096, 2048
P = 128
KD, KF = D // P, F // P  # 8, 32

_NC_CACHE: dict = {}


def _plan_capacity(max_count: int):
    """Pick capacity C = n_chunks * L with L <= 512, L a multiple of 16."""
    max_count = max(max_count, 16)
    n = -(-max_count // 512)  # ceil
    L = -(-max_count // (n * 16)) * 16
    return n * L, L, n


def _build_nc(C: int, L: int):
    nchunks = C // L
    # Bacc (not raw Bass): its finalize() runs move_matmul_waits_to_ldweights
    # + generate_event_semaphores, which split multi-wait instructions down to
    # the TRN2 limit of 1 sync wait per instruction.
    nc = bacc.Bacc(None)
    # x packed per chunk:   x_pack[c, p, k, t]  = x_e.T[k*P + p, c*L + t]
    # w1 packed per f-tile: w1_pack[f, p, k, c] = w1[e][f*P + c, k*P + p]
    # w2 packed per d-tile: w2_pack[d, p, k2, c] = w2[e][d*P + c, k2*P + p]
    x_in = nc.declare_dram_parameter("x_pack", [nchunks, P, KD, L], mybir.dt.bfloat16, isOutput=False)
    # w1 split: f-tile 0 alone (unblocks the first matmul group asap), then
    # 31 f-tiles in 4 grouped loads (16KB contiguous lines, fewer semaphores).
    w1_in0 = nc.declare_dram_parameter("w1_pack0", [P, KD, P], mybir.dt.bfloat16, isOutput=False)
    W1G = [8, 8, 8, 7]
    w1_ing = [
        nc.declare_dram_parameter(f"w1_packg{g}", [P, W1G[g], KD, P], mybir.dt.bfloat16, isOutput=False)
        for g in range(4)
    ]
    cf_in = nc.declare_dram_parameter("coef", [P, C], mybir.dt.float32, isOutput=False)
    W2G = [4, 4]
    w2_ing = [
        nc.declare_dram_parameter(f"w2_packg{g}", [P, W2G[g], KF, P], mybir.dt.bfloat16, isOutput=False)
        for g in range(2)
    ]
    y_out = nc.declare_dram_parameter("y_dc", [KD, P, C], mybir.dt.bfloat16, isOutput=True)

    with tile.TileContext(nc) as tc:
        with (
            tc.tile_pool(name="wpool", bufs=1) as wpool,
            tc.tile_pool(name="apool", bufs=1) as apool,
            tc.tile_pool(name="ps1", bufs=4, space="PSUM") as ps1,
            tc.tile_pool(name="ps2", bufs=3, space="PSUM") as ps2,
            tc.tile_pool(name="psw", bufs=1, space="PSUM") as psw,
            tc.tile_pool(name="ypool", bufs=6) as ypool,
        ):
            # ---- PE warm-up: ~4us of dependency-free matmuls run during the
            # input-DMA head so the HAM clock gate is at 8/8 (2.4 GHz) when
            # the real stream starts (saves ~3us of cold-clock matmuls).
            warm = wpool.tile([P, P], mybir.dt.bfloat16, name="warm")
            nc.vector.memset(warm, 0.0)
            # clock ramp saturates after ~3us of sustained PE activity; 80
            # 64-col matmuls at the ramping clock cover that while the first
            # input DMAs land.
            N_WARM = 80
            ps_w = psw.tile([P, 64], mybir.dt.float32, name="ps_w")
            for i in range(N_WARM):
                nc.tensor.matmul(
                    ps_w, warm, warm[:, :64], start=(i == 0), stop=(i == N_WARM - 1)
                )
            # ---- loads, in consumption order: w1_0 + x unblock stage 1.
            x_sb = [None] * nchunks

            w1_sb0 = wpool.tile([P, KD, P], mybir.dt.bfloat16, name="w1_0")
            nc.sync.dma_start(w1_sb0, w1_in0[:])
            for c in range(nchunks):
                xt = apool.tile([P, KD, L], mybir.dt.bfloat16, name=f"x_{c}")
                nc.sync.dma_start(xt, x_in[c])
                x_sb[c] = xt
            w1_sbg = []
            for g in range(4):
                wt = wpool.tile([P, W1G[g], KD, P], mybir.dt.bfloat16, name=f"w1_g{g}")
                nc.sync.dma_start(wt, w1_ing[g][:])
                w1_sbg.append(wt)
            # w1_sb[f] views: f=0 standalone, rest from the 4 groups
            w1_sb = [w1_sb0]
            for g in range(4):
                for j in range(W1G[g]):
                    w1_sb.append(w1_sbg[g][:, j])
            coef_sb = apool.tile([P, C], mybir.dt.float32, name="coef_sb")
            nc.sync.dma_start(coef_sb, cf_in[:])
            w2_sbg = []
            for g in range(2):
                wt = wpool.tile([P, W2G[g], KF, P], mybir.dt.bfloat16, name=f"w2_g{g}")
                nc.sync.dma_start(wt, w2_ing[g][:])
                w2_sbg.append(wt)
            w2_sb = [w2_sbg[d // 4][:, d % 4] for d in range(KD)]

            h_sb = [
                apool.tile([P, C], mybir.dt.bfloat16, name=f"h_{k2}") for k2 in range(KF)
            ]

            # ---- stage 1: h.T[f*P:(f+1)*P, :] = gelu(w1 @ x.T) per F-tile
            for f in range(KF):
                for c in range(nchunks):
                    c0 = c * L
                    ps = ps1.tile([P, L], mybir.dt.float32, name="ps1t", tag="ps1t")
                    for k in range(KD):
                        nc.tensor.matmul(
                            ps,
                            w1_sb[f][:, k],
                            x_sb[c][:, k],
                            start=(k == 0),
                            stop=(k == KD - 1),
                        )
                    nc.scalar.activation(
                        out=h_sb[f][:, c0 : c0 + L],
                        in_=ps,
                        func=mybir.ActivationFunctionType.Gelu,
                    )

            # ---- stage 2: y.T[d*P:(d+1)*P, :] = (w2 @ h.T) * coef
            # Both chunks collect into one bf16 staging tile; a single DMA
            # per d-tile keeps line sizes up and sync count down.
            for d in range(KD):
                y_sb = ypool.tile([P, C], mybir.dt.bfloat16, name="y_sb", tag="y_sb")
                for c in range(nchunks):
                    c0 = c * L
                    ps = ps2.tile([P, L], mybir.dt.float32, name="ps2t", tag="ps2t")
                    for k2 in range(KF):
                        nc.tensor.matmul(
                            ps,
                            w2_sb[d][:, k2],
                            h_sb[k2][:, c0 : c0 + L],
                            start=(k2 == 0),
                            stop=(k2 == KF - 1),
                        )
                    nc.vector.tensor_mul(y_sb[:, c0 : c0 + L], ps, coef_sb[:, c0 : c0 + L])
                nc.sync.dma_start(y_out[d], y_sb)
    nc.finalize()
    return nc


def _route(x: np.ndarray, gate_w: np.ndarray):
    """fp64 gating; matches reference fp32 routing (selection gaps >> fp32 eps)."""
    logits = x.astype(np.float64) @ gate_w.astype(np.float64).T  # [N, E]
    top2 = np.argsort(-logits, axis=1, kind="stable")[:, :2]  # [N, 2]
    v = np.take_along_axis(logits, top2, axis=1)
    v = v - v.max(axis=1, keepdims=True)
    ew = np.exp(v)
    w = ew / ew.sum(axis=1, keepdims=True)  # [N, 2]
    return top2, w.astype(np.float32)


def _run(inputs: dict, trace: bool = False, trace_cores=None):
    x = np.asarray(inputs["x"], dtype=np.float32)
    gate_w = np.asarray(inputs["gate_w"], dtype=np.float32)
    w1 = np.asarray(inputs["w1"], dtype=np.float32)
    w2 = np.asarray(inputs["w2"], dtype=np.float32)
    n = x.shape[0]

    top2, wsm = _route(x, gate_w)

    idx_list, coef_list = [], []
    for e in range(E):
        mask = top2 == e  # [N, 2]
        sel = mask.any(axis=1)
        idx = np.nonzero(sel)[0]
        we = np.where(mask[idx, 0], wsm[idx, 0], wsm[idx, 1])
        idx_list.append(idx)
        coef_list.append(we.astype(np.float32))

    max_count = max(len(i) for i in idx_list)
    C, L, nchunks = _plan_capacity(max_count)

    key = (C, L)
    if key not in _NC_CACHE:
        _NC_CACHE[key] = _build_nc(C, L)
    nc = _NC_CACHE[key]

    in_maps = []
    for e in range(E):
        idx, cf = idx_list[e], coef_list[e]
        cnt = len(idx)
        xe = np.zeros((D, C), dtype=BF16)
        xe[:, :cnt] = x[idx].T.astype(BF16)
        # [D, C] -> [k, p, c, t] -> pack [c, p, k, t]
        x_pack = np.ascontiguousarray(
            xe.reshape(KD, P, nchunks, L).transpose(2, 1, 0, 3)
        )
        coef = np.zeros((C,), dtype=np.float32)
        coef[:cnt] = cf
        coef_rep = np.ascontiguousarray(np.broadcast_to(coef, (P, C)))
        # w1[e] is [F, D]: [f, c, k, p] -> [f, p, k, c]; f=0 standalone, the
        # rest regrouped partition-major: [p, f_in_group, k, c]
        w1_pack = w1[e].astype(BF16).reshape(KF, P, KD, P).transpose(0, 3, 2, 1)
        w2_pack = w2[e].astype(BF16).reshape(KD, P, KF, P).transpose(0, 3, 2, 1)
        m = {
            "x_pack": x_pack,
            "coef": coef_rep,
            "w1_pack0": np.ascontiguousarray(w1_pack[0]),
        }
        W1G = [8, 8, 8, 7]
        f0 = 1
        for g in range(4):
            m[f"w1_packg{g}"] = np.ascontiguousarray(
                w1_pack[f0 : f0 + W1G[g]].transpose(1, 0, 2, 3)
            )
            f0 += W1G[g]
        for g in range(2):
            m[f"w2_packg{g}"] = np.ascontiguousarray(
                w2_pack[4 * g : 4 * g + 4].transpose(1, 0, 2, 3)
            )
        in_maps.append(m)

    res = run_bass_kernel_spmd(
        nc,
        in_maps,
        list(range(E)),
        trace=trace,
        trace_cores=trace_cores,
    )

    out = np.zeros((n, D), dtype=np.float32)
    for e in range(E):
        idx = idx_list[e]
        cnt = len(idx)
        y_dc = np.asarray(res.results[e]["y_dc"]).astype(np.float32)  # [KD, P, C] bf16
        y = y_dc.reshape(D, C)[:, :cnt]  # [D, cnt]
        out[idx] += y.T
    return out, res


def kernel(**inputs) -> np.ndarray:
    out, _ = _run(inputs, trace=False)
    return out


if __name__ == "__main__":
    rng = np.random.default_rng(0)
    fake = {
        "x": rng.standard_normal((NTOK, D), dtype=np.float32),
        "gate_w": (rng.standard_normal((E, D)) * 0.02).astype(np.float32),
        "w1": (rng.standard_normal((E, F, D)) * 0.02).astype(np.float32),
        "w2": (rng.standard_normal((E, D, F)) * 0.02).astype(np.float32),
    }
    out = kernel(**fake)
    print("ok", out.shape, out.dtype, np.abs(out).max())



# revision 9
# speedup vs baseline: 1.4618x; 1.0881x over previous
"""MoE (top-2 routing, E=8 experts) Trainium2 kernel.

Strategy (expert-parallel across 8 NeuronCores):
  - Host: gate (tiny: [2048,1024]@[1024,8]) in fp64, top-2 + softmax exactly
    reproducing the reference routing (selection gaps are ~4e-4, far above
    fp32 noise, so fp64 routing == reference fp32 routing).
  - Host: dispatch — gather each expert's tokens into a fixed-capacity,
    zero-padded buffer; one expert per core. Weights are pre-transposed,
    cast to bf16 and packed per matmul-group on host so each core's full
    working set (w1.T 8MiB + w2.T 8MiB bf16) is SBUF-resident and every
    DMA is contiguous per partition and arrives in consumption order.
  - Device (per core): h.T = gelu(w1 @ x.T) [F x C], y.T = w2 @ h.T [D x C],
    tokens live in the matmul free dimension. Routing coefficient is applied
    as part of the PSUM->SBUF eviction (DVE multiply). fp32 accumulation in
    PSUM throughout.
  - Host: combine — scatter-add the two expert contributions per token.
"""

import numpy as np
import ml_dtypes

import sys

if "/opt/trn_rl_repo" not in sys.path:
    sys.path.insert(0, "/opt/trn_rl_repo")

import concourse.tile as tile
from concourse import bacc, mybir
from concourse.bass_utils import run_bass_kernel_spmd

BF16 = ml_dtypes.bfloat16

E, D, F, NTOK = 8, 1024, 4096, 2048
P = 128
KD, KF = D // P, F // P  # 8, 32

_NC_CACHE: dict = {}


def _plan_capacity(max_count: int):
    """Pick capacity C = n_chunks * L with L <= 512, L a multiple of 16."""
    max_count = max(max_count, 16)
    n = -(-max_count // 512)  # ceil
    L = -(-max_count // (n * 16)) * 16
    return n * L, L, n


def _build_nc(C: int, L: int):
    nchunks = C // L
    # Bacc (not raw Bass): its finalize() runs move_matmul_waits_to_ldweights
    # + generate_event_semaphores, which split multi-wait instructions down to
    # the TRN2 limit of 1 sync wait per instruction.
    nc = bacc.Bacc(None)
    # x packed per chunk:   x_pack[c, p, k, t]  = x_e.T[k*P + p, c*L + t]
    # w1 packed per f-tile: w1_pack[f, p, k, c] = w1[e][f*P + c, k*P + p]
    # w2 packed per d-tile: w2_pack[d, p, k2, c] = w2[e][d*P + c, k2*P + p]
    x_in = nc.declare_dram_parameter("x_pack", [nchunks, P, KD, L], mybir.dt.bfloat16, isOutput=False)
    w1_in = nc.declare_dram_parameter("w1_pack", [KF, P, KD, P], mybir.dt.bfloat16, isOutput=False)
    w2_in = nc.declare_dram_parameter("w2_pack", [KD, P, KF, P], mybir.dt.bfloat16, isOutput=False)
    cf_in = nc.declare_dram_parameter("coef", [P, C], mybir.dt.float32, isOutput=False)
    y_out = nc.declare_dram_parameter("y_dc", [KD, P, C], mybir.dt.bfloat16, isOutput=True)

    with tile.TileContext(nc) as tc:
        with (
            tc.tile_pool(name="wpool", bufs=1) as wpool,
            tc.tile_pool(name="apool", bufs=1) as apool,
            tc.tile_pool(name="ps1", bufs=4, space="PSUM") as ps1,
            tc.tile_pool(name="ps2", bufs=3, space="PSUM") as ps2,
            tc.tile_pool(name="psw", bufs=1, space="PSUM") as psw,
            tc.tile_pool(name="ypool", bufs=6) as ypool,
        ):
            # ---- PE warm-up: ~4us of dependency-free matmuls run during the
            # input-DMA head so the HAM clock gate is at 8/8 (2.4 GHz) when
            # the real stream starts (saves ~3us of cold-clock matmuls).
            warm = wpool.tile([P, P], mybir.dt.bfloat16, name="warm")
            nc.vector.memset(warm, 0.0)
            N_WARM = 112
            ps_w = psw.tile([P, 64], mybir.dt.float32, name="ps_w")
            for i in range(N_WARM):
                nc.tensor.matmul(
                    ps_w, warm, warm[:, :64], start=(i == 0), stop=(i == N_WARM - 1)
                )
            # ---- loads, in consumption order, one DMA per consumer group.
            w1_sb = [None] * KF
            x_sb = [None] * nchunks

            def load_w1(f):
                wt = wpool.tile([P, KD, P], mybir.dt.bfloat16, name=f"w1_{f}")
                nc.sync.dma_start(wt, w1_in[f])
                w1_sb[f] = wt

            load_w1(0)
            for c in range(nchunks):
                xt = apool.tile([P, KD, L], mybir.dt.bfloat16, name=f"x_{c}")
                nc.sync.dma_start(xt, x_in[c])
                x_sb[c] = xt
            for f in range(1, KF):
                load_w1(f)
            coef_sb = apool.tile([P, C], mybir.dt.float32, name="coef_sb")
            nc.sync.dma_start(coef_sb, cf_in[:])
            w2_sb = []
            for d in range(KD):
                wt = wpool.tile([P, KF, P], mybir.dt.bfloat16, name=f"w2_{d}")
                nc.sync.dma_start(wt, w2_in[d])
                w2_sb.append(wt)

            h_sb = [
                apool.tile([P, C], mybir.dt.bfloat16, name=f"h_{k2}") for k2 in range(KF)
            ]

            # ---- stage 1: h.T[f*P:(f+1)*P, :] = gelu(w1 @ x.T) per F-tile
            for f in range(KF):
                for c in range(nchunks):
                    c0 = c * L
                    ps = ps1.tile([P, L], mybir.dt.float32, name="ps1t", tag="ps1t")
                    for k in range(KD):
                        nc.tensor.matmul(
                            ps,
                            w1_sb[f][:, k],
                            x_sb[c][:, k],
                            start=(k == 0),
                            stop=(k == KD - 1),
                        )
                    nc.scalar.activation(
                        out=h_sb[f][:, c0 : c0 + L],
                        in_=ps,
                        func=mybir.ActivationFunctionType.Gelu,
                    )

            # ---- stage 2: y.T[d*P:(d+1)*P, :] = (w2 @ h.T) * coef
            # Both chunks collect into one bf16 staging tile; a single DMA
            # per d-tile keeps line sizes up and sync count down.
            for d in range(KD):
                y_sb = ypool.tile([P, C], mybir.dt.bfloat16, name="y_sb", tag="y_sb")
                for c in range(nchunks):
                    c0 = c * L
                    ps = ps2.tile([P, L], mybir.dt.float32, name="ps2t", tag="ps2t")
                    for k2 in range(KF):
                        nc.tensor.matmul(
                            ps,
                            w2_sb[d][:, k2],
                            h_sb[k2][:, c0 : c0 + L],
                            start=(k2 == 0),
                            stop=(k2 == KF - 1),
                        )
                    nc.vector.tensor_mul(y_sb[:, c0 : c0 + L], ps, coef_sb[:, c0 : c0 + L])
                nc.sync.dma_start(y_out[d], y_sb)
    nc.finalize()
    return nc


def _route(x: np.ndarray, gate_w: np.ndarray):
    """fp64 gating; matches reference fp32 routing (selection gaps >> fp32 eps)."""
    logits = x.astype(np.float64) @ gate_w.astype(np.float64).T  # [N, E]
    top2 = np.argsort(-logits, axis=1, kind="stable")[:, :2]  # [N, 2]
    v = np.take_along_axis(logits, top2, axis=1)
    v = v - v.max(axis=1, keepdims=True)
    ew = np.exp(v)
    w = ew / ew.sum(axis=1, keepdims=True)  # [N, 2]
    return top2, w.astype(np.float32)


def _run(inputs: dict, trace: bool = False, trace_cores=None):
    x = np.asarray(inputs["x"], dtype=np.float32)
    gate_w = np.asarray(inputs["gate_w"], dtype=np.float32)
    w1 = np.asarray(inputs["w1"], dtype=np.float32)
    w2 = np.asarray(inputs["w2"], dtype=np.float32)
    n = x.shape[0]

    top2, wsm = _route(x, gate_w)

    idx_list, coef_list = [], []
    for e in range(E):
        mask = top2 == e  # [N, 2]
        sel = mask.any(axis=1)
        idx = np.nonzero(sel)[0]
        we = np.where(mask[idx, 0], wsm[idx, 0], wsm[idx, 1])
        idx_list.append(idx)
        coef_list.append(we.astype(np.float32))

    max_count = max(len(i) for i in idx_list)
    C, L, nchunks = _plan_capacity(max_count)

    key = (C, L)
    if key not in _NC_CACHE:
        _NC_CACHE[key] = _build_nc(C, L)
    nc = _NC_CACHE[key]

    in_maps = []
    for e in range(E):
        idx, cf = idx_list[e], coef_list[e]
        cnt = len(idx)
        xe = np.zeros((D, C), dtype=BF16)
        xe[:, :cnt] = x[idx].T.astype(BF16)
        # [D, C] -> [k, p, c, t] -> pack [c, p, k, t]
        x_pack = np.ascontiguousarray(
            xe.reshape(KD, P, nchunks, L).transpose(2, 1, 0, 3)
        )
        coef = np.zeros((C,), dtype=np.float32)
        coef[:cnt] = cf
        coef_rep = np.ascontiguousarray(np.broadcast_to(coef, (P, C)))
        # w1[e] is [F, D]: [f, c, k, p] -> pack [f, p, k, c]
        w1_pack = np.ascontiguousarray(
            w1[e].astype(BF16).reshape(KF, P, KD, P).transpose(0, 3, 2, 1)
        )
        # w2[e] is [D, F]: [d, c, k2, p] -> pack [d, p, k2, c]
        w2_pack = np.ascontiguousarray(
            w2[e].astype(BF16).reshape(KD, P, KF, P).transpose(0, 3, 2, 1)
        )
        in_maps.append(
            {
                "x_pack": x_pack,
                "w1_pack": w1_pack,
                "w2_pack": w2_pack,
                "coef": coef_rep,
            }
        )

    res = run_bass_kernel_spmd(
        nc,
        in_maps,
        list(range(E)),
        trace=trace,
        trace_cores=trace_cores,
    )

    out = np.zeros((n, D), dtype=np.float32)
    for e in range(E):
        idx = idx_list[e]
        cnt = len(idx)
        y_dc = np.asarray(res.results[e]["y_dc"]).astype(np.float32)  # [KD, P, C] bf16
        y = y_dc.reshape(D, C)[:, :cnt]  # [D, cnt]
        out[idx] += y.T
    return out, res


def kernel(**inputs) -> np.ndarray:
    out, _ = _run(inputs, trace=False)
    return out


if __name__ == "__main__":
    rng = np.random.default_rng(0)
    fake = {
        "x": rng.standard_normal((NTOK, D), dtype=np.float32),
        "gate_w": (rng.standard_normal((E, D)) * 0.02).astype(np.float32),
        "w1": (rng.standard_normal((E, F, D)) * 0.02).astype(np.float32),
        "w2": (rng.standard_normal((E, D, F)) * 0.02).astype(np.float32),
    }
    out = kernel(**fake)
    print("ok", out.shape, out.dtype, np.abs(out).max())



# revision 10
# speedup vs baseline: 1.4810x; 1.0131x over previous
"""MoE top-2 Trainium2 kernel — 2-way expert-F-split for load balance.

Each expert's FFN hidden dim F is split in half across two cores; each
core hosts halves of TWO experts, paired big-with-small by token count
(sorted desc x asc), so per-core columns = cnt_big + cnt_small ~= 1032
instead of the expert-parallel worst case 544+. Per-column work halves
(F/2), so PE cycles = 256 * (capA + capB) ~= 270k vs 279k baseline, with
identical total DMA traffic (w1/w2 slices are the same byte count).

Device output is the coef-weighted PARTIAL y (over the core's F-half) in
bf16; the host sums the two halves of each expert and scatters.
"""

import numpy as np
import ml_dtypes

import sys

if "/opt/trn_rl_repo" not in sys.path:
    sys.path.insert(0, "/opt/trn_rl_repo")

import concourse.tile as tile
from concourse import bacc, mybir
from concourse.bass_utils import run_bass_kernel_spmd

BF16 = ml_dtypes.bfloat16

E, D, F, NTOK = 8, 1024, 4096, 2048
P = 128
KD = D // P          # 8 k-tiles over D
KFL = F // 2 // P    # 16 f-tiles per expert half
NCORE = 8

_NC_CACHE: dict = {}


def _plan_capacity(max_count: int, max_l: int = 512):
    max_count = max(max_count, 16)
    n = -(-max_count // max_l)
    L = -(-max_count // (n * 16)) * 16
    return n * L, L, n


def _build_nc(CA: int, LA: int, CB: int, LB: int):
    nA, nB = CA // LA, CB // LB
    C = CA + CB
    # chunk list: (slot, within-slot chunk idx, global col offset, width).
    # A first: stage-2 d0 then depends on h_a (ready mid-stage-1), not on the
    # very last stage-1 tile — keeps the PE stream gapless at the seam.
    chunks = [("a", c, c * LA, LA) for c in range(nA)] + [
        ("b", c, CA + c * LB, LB) for c in range(nB)
    ]
    # stage-1 stationary consumption order: slot a f0..15, then slot b f0..15
    order1 = [("a", f) for f in range(KFL)] + [("b", f) for f in range(KFL)]
    # stage-2 stationary consumption order: (a,d0),(b,d0),(a,d1),...
    order2 = [(s, d) for d in range(KD) for s in ("a", "b")]

    nc = bacc.Bacc(None)
    xa_in = nc.declare_dram_parameter("xa_pack", [nA, P, KD, LA], mybir.dt.bfloat16, isOutput=False)
    xb_in = nc.declare_dram_parameter("xb_pack", [nB, P, KD, LB], mybir.dt.bfloat16, isOutput=False)
    # w1 tiles in order1 order; w2 tiles in order2 order. Fine-grained per-tile
    # DMAs: each 256KB tile unblocks its consumer as soon as it lands (grouped
    # multi-MB loads starve the PE early — and a stalled PE drops its clock).
    w1_in = nc.declare_dram_parameter("w1_pack", [2 * KFL, P, KD, P], mybir.dt.bfloat16, isOutput=False)
    cf_in = nc.declare_dram_parameter("coef", [P, C], mybir.dt.float32, isOutput=False)
    w2_in = nc.declare_dram_parameter("w2_pack", [2 * KD, P, KFL, P], mybir.dt.bfloat16, isOutput=False)
    y_out = nc.declare_dram_parameter("y_dc", [KD, P, C], mybir.dt.bfloat16, isOutput=True)

    with tile.TileContext(nc) as tc:
        with (
            tc.tile_pool(name="wpool", bufs=1) as wpool,
            tc.tile_pool(name="apool", bufs=1) as apool,
            tc.tile_pool(name="ps1", bufs=2, space="PSUM") as ps1,
            tc.tile_pool(name="ps2", bufs=2, space="PSUM") as ps2,
            tc.tile_pool(name="ypool", bufs=4) as ypool,
        ):
            # PE clock warm-up during the DMA head; borrows a ps1 slot-a
            # buffer, never read.
            warm = wpool.tile([P, P], mybir.dt.bfloat16, name="warm")
            nc.vector.memset(warm, 0.0)
            # warmup length is tuned to the sustained w1 DMA stream rate, not
            # just first-data: a shorter warmup starts the stream before the
            # per-f weight DMAs can keep up, and a single mid-stream stall
            # drops the PE clock for tens of microseconds.
            N_WARM = 112
            ps_w = ps1.tile([P, LA], mybir.dt.float32, name="ps1t_a", tag="ps1t_a")
            for i in range(N_WARM):
                nc.tensor.matmul(
                    ps_w[:, :64], warm, warm[:, :64], start=(i == 0), stop=(i == N_WARM - 1)
                )
            # ---- loads in consumption order: w1_a0, xa (both chunks), rest
            # of w1_a, then xb just before w1_b, coef, w2 per-tile.
            w1_sb = {}
            xa_sb, xb_sb = [], []

            def load_w1(i):
                wt = wpool.tile([P, KD, P], mybir.dt.bfloat16, name=f"w1_{i}")
                nc.sync.dma_start(wt, w1_in[i])
                w1_sb[order1[i]] = wt

            load_w1(0)
            for c in range(nA):
                xt = apool.tile([P, KD, LA], mybir.dt.bfloat16, name=f"xa_{c}")
                nc.sync.dma_start(xt, xa_in[c])
                xa_sb.append(xt)
            for i in range(1, KFL):
                load_w1(i)
            for c in range(nB):
                xt = apool.tile([P, KD, LB], mybir.dt.bfloat16, name=f"xb_{c}")
                nc.sync.dma_start(xt, xb_in[c])
                xb_sb.append(xt)
            for i in range(KFL, 2 * KFL):
                load_w1(i)
            coef_sb = apool.tile([P, C], mybir.dt.float32, name="coef_sb")
            nc.sync.dma_start(coef_sb, cf_in[:])
            w2_sb = {}
            for i in range(2 * KD):
                wt = wpool.tile([P, KFL, P], mybir.dt.bfloat16, name=f"w2_{i}")
                nc.sync.dma_start(wt, w2_in[i])
                w2_sb[order2[i]] = wt

            h_sb = {
                "a": apool.tile([P, KFL, CA], mybir.dt.bfloat16, name="h_a"),
                "b": apool.tile([P, KFL, CB], mybir.dt.bfloat16, name="h_b"),
            }
            x_sb = {"a": xa_sb, "b": xb_sb}

            # ---- stage 1: h_s[f] = gelu(w1_s[f] @ x_s.T)
            for s, f in order1:
                for c, (_, ci, _, L) in enumerate(
                    [ch for ch in chunks if ch[0] == s]
                ):
                    ps = ps1.tile([P, LA if s == "a" else LB], mybir.dt.float32,
                                  name=f"ps1t_{s}", tag=f"ps1t_{s}")
                    for k in range(KD):
                        nc.tensor.matmul(
                            ps,
                            w1_sb[(s, f)][:, k],
                            x_sb[s][ci][:, k],
                            start=(k == 0),
                            stop=(k == KD - 1),
                        )
                    c0 = ci * (LA if s == "a" else LB)
                    nc.scalar.activation(
                        out=h_sb[s][:, f, c0 : c0 + (LA if s == "a" else LB)],
                        in_=ps,
                        func=mybir.ActivationFunctionType.Gelu,
                    )

            # ---- stage 2: y_part.T = (w2_s @ h_s.T) * coef, one DMA per d
            for d in range(KD):
                y_sb = ypool.tile([P, C], mybir.dt.bfloat16, name="y_sb", tag="y_sb")
                for s, ci, g0, L in chunks:
                    c0 = ci * L
                    ps = ps2.tile([P, L], mybir.dt.float32,
                                  name=f"ps2t_{s}", tag=f"ps2t_{s}")
                    for k2 in range(KFL):
                        nc.tensor.matmul(
                            ps,
                            w2_sb[(s, d)][:, k2],
                            h_sb[s][:, k2, c0 : c0 + L],
                            start=(k2 == 0),
                            stop=(k2 == KFL - 1),
                        )
                    nc.vector.tensor_mul(
                        y_sb[:, g0 : g0 + L], ps, coef_sb[:, g0 : g0 + L]
                    )
                nc.sync.dma_start(y_out[d], y_sb)
    nc.finalize()
    return nc


def _route(x: np.ndarray, gate_w: np.ndarray):
    logits = x.astype(np.float64) @ gate_w.astype(np.float64).T
    top2 = np.argsort(-logits, axis=1, kind="stable")[:, :2]
    v = np.take_along_axis(logits, top2, axis=1)
    v = v - v.max(axis=1, keepdims=True)
    ew = np.exp(v)
    w = ew / ew.sum(axis=1, keepdims=True)
    return top2, w.astype(np.float32)


def _pack_x(xe: np.ndarray, nch: int, L: int):
    """[D, C] fp32 -> [nch, P, KD, L] bf16 (zero-padded already)."""
    return np.ascontiguousarray(
        xe.astype(BF16).reshape(KD, P, nch, L).transpose(2, 1, 0, 3)
    )


def _run(inputs: dict, trace: bool = False, trace_cores=None):
    x = np.asarray(inputs["x"], dtype=np.float32)
    gate_w = np.asarray(inputs["gate_w"], dtype=np.float32)
    w1 = np.asarray(inputs["w1"], dtype=np.float32)
    w2 = np.asarray(inputs["w2"], dtype=np.float32)
    n = x.shape[0]

    top2, wsm = _route(x, gate_w)

    idx_list, coef_list = [], []
    for e in range(E):
        mask = top2 == e
        idx = np.nonzero(mask.any(axis=1))[0]
        we = np.where(mask[idx, 0], wsm[idx, 0], wsm[idx, 1])
        idx_list.append(idx)
        coef_list.append(we.astype(np.float32))

    counts = np.array([len(i) for i in idx_list])
    order = np.argsort(-counts, kind="stable")
    pairs = [(int(order[m]), int(order[7 - m])) for m in range(4)]  # (big, small)
    maxA = max(counts[a] for a, _ in pairs)
    maxB = max(counts[b] for _, b in pairs)
    CA, LA, nA = _plan_capacity(int(maxA))
    CB, LB, nB = _plan_capacity(int(maxB))
    C = CA + CB

    key = (CA, LA, CB, LB)
    if key not in _NC_CACHE:
        _NC_CACHE[key] = _build_nc(CA, LA, CB, LB)
    nc = _NC_CACHE[key]

    in_maps = [None] * NCORE
    for m, (ea, eb) in enumerate(pairs):
        # x / coef shared by both halves
        xea = np.zeros((D, CA), dtype=np.float32)
        xea[:, : counts[ea]] = x[idx_list[ea]].T
        xeb = np.zeros((D, CB), dtype=np.float32)
        xeb[:, : counts[eb]] = x[idx_list[eb]].T
        xa_pack = _pack_x(xea, nA, LA)
        xb_pack = _pack_x(xeb, nB, LB)
        coef = np.zeros((C,), dtype=np.float32)
        coef[: counts[ea]] = coef_list[ea]
        coef[CA : CA + counts[eb]] = coef_list[eb]
        coef_rep = np.ascontiguousarray(np.broadcast_to(coef, (P, C)))
        for s in range(2):
            half = slice(s * (F // 2), (s + 1) * (F // 2))
            # stage-1 stationary tiles in order1: a f0..15 then b f0..15
            # w1[e][half] is [F/2, D]: [f, c, k, p] -> [f, p, k, c]
            w1a = w1[ea][half].astype(BF16).reshape(KFL, P, KD, P).transpose(0, 3, 2, 1)
            w1b = w1[eb][half].astype(BF16).reshape(KFL, P, KD, P).transpose(0, 3, 2, 1)
            w1_flat = np.ascontiguousarray(np.concatenate([w1a, w1b]))  # [32, P, KD, P]
            # stage-2 stationary in order2: (a,d),(b,d) per d
            # w2[e][:, half] is [D, F/2]: [d, c, k2, p] -> [d, p, k2, c]
            w2a = w2[ea][:, half].astype(BF16).reshape(KD, P, KFL, P).transpose(0, 3, 2, 1)
            w2b = w2[eb][:, half].astype(BF16).reshape(KD, P, KFL, P).transpose(0, 3, 2, 1)
            w2_flat = np.empty((2 * KD, P, KFL, P), dtype=BF16)
            w2_flat[0::2] = w2a
            w2_flat[1::2] = w2b
            in_maps[2 * m + s] = {
                "xa_pack": xa_pack,
                "xb_pack": xb_pack,
                "coef": coef_rep,
                "w1_pack": w1_flat,
                "w2_pack": np.ascontiguousarray(w2_flat),
            }

    res = run_bass_kernel_spmd(
        nc,
        in_maps,
        list(range(NCORE)),
        trace=trace,
        trace_cores=trace_cores,
    )

    out = np.zeros((n, D), dtype=np.float32)
    for m, (ea, eb) in enumerate(pairs):
        y0 = np.asarray(res.results[2 * m]["y_dc"]).astype(np.float32)
        y1 = np.asarray(res.results[2 * m + 1]["y_dc"]).astype(np.float32)
        ys = (y0 + y1).reshape(D, C)
        out[idx_list[ea]] += ys[:, : counts[ea]].T
        out[idx_list[eb]] += ys[:, CA : CA + counts[eb]].T
    return out, res


def kernel(**inputs) -> np.ndarray:
    out, _ = _run(inputs, trace=False)
    return out


if __name__ == "__main__":
    rng = np.random.default_rng(0)
    fake = {
        "x": rng.standard_normal((NTOK, D), dtype=np.float32),
        "gate_w": (rng.standard_normal((E, D)) * 0.02).astype(np.float32),
        "w1": (rng.standard_normal((E, F, D)) * 0.02).astype(np.float32),
        "w2": (rng.standard_normal((E, D, F)) * 0.02).astype(np.float32),
    }
    out = kernel(**fake)
    print("ok", out.shape, out.dtype, np.abs(out).max())
